# revision 11
# baseline (speedup 1.0000x reference)
"""Trainium2 Bass kernel for CausalWanSelfAttention (KV-cache-bias attention).

Math: the reference's disjoint-segment attention + LSE merge is exactly
global softmax with a per-key bias b_l (log 0.1 on keys in
[frame_seqlen, current_block_start)).  Since exp(s + b_l) = exp(s) * w_l,
the bias folds into V host-side:  out = (E @ [w*V | w]) -> normalize,
where E = exp(S) needs no max-subtraction (scores ~ N(0,1), max ~ 6).

Sharding: 24 units = (head h in 0..11, q-half in {0,1}), 3 units per core.
Each unit: 1024 queries x 1 head x all 8192 keys.  K/V slices are
duplicated per-unit host-side so the device program is uniform SPMD.

Device layout per unit (all matmuls bf16, accumulate fp32 PSUM):
  phase A: S^T[lchunk 128, q 1024] = K^T-chunk.T @ Q^T      (2 MMs N=512)
  exp:     E = exp(S^T * scale) on ACT, bf16 out            (1 instr N=1024)
  phase B: O[q 128, 129] += E-slice.T @ Vaug-chunk          (8 MMs N=129)
  epilogue: out[q, d] = O[:, 0:128] * (1 / O[:, 128])       (DVE)
"""

import math
import sys

for _p in ("/opt/trn_rl_repo",):
    if _p not in sys.path:
        sys.path.insert(0, _p)

import numpy as np
import ml_dtypes

import concourse.bass as bass
import concourse.mybir as mybir
import concourse.tile as tile
from concourse import bacc
from concourse.bass_utils import run_bass_kernel_spmd

BF16 = mybir.dt.bfloat16
F32 = mybir.dt.float32
NP_BF16 = ml_dtypes.bfloat16

B, LQ, LK, H, D = 1, 2048, 8192, 12, 128
N_CORES = 8
UNITS_PER_CORE = 3          # 24 units = 12 heads x 2 q-halves
QSPAN = 1024                # queries per unit
NQB = QSPAN // 128          # 8 q-blocks of 128 per unit
NLC = LK // 128             # 64 key chunks of 128
VPAD = 256                  # v-aug row padded to 256 bf16 (512B) for DMA
SCALE = 1.0 / math.sqrt(D)

_CACHED = None  # (nc, names)


def _build_program():
    nc = bacc.Bacc("TRN2", target_bir_lowering=False, debug=False,
                   enable_asserts=False)

    qt_d = nc.dram_tensor("qt", [UNITS_PER_CORE, 128, QSPAN], BF16,
                          kind="ExternalInput")
    kt_d = nc.dram_tensor("kt", [UNITS_PER_CORE, 128, LK], BF16,
                          kind="ExternalInput")
    va_d = nc.dram_tensor("va", [UNITS_PER_CORE, LK, VPAD], BF16,
                          kind="ExternalInput")
    out_d = nc.dram_tensor("out", [UNITS_PER_CORE, QSPAN, 128], F32,
                           kind="ExternalOutput")

    qt_ap = qt_d.ap()
    kt_ap = kt_d.ap()
    # [u, (c p), j] -> [u, p, c, j]: partition = l within chunk
    va_ap = va_d.ap().rearrange("u (c p) j -> u p c j", p=128)
    out_ap = out_d.ap()

    with tile.TileContext(nc) as tc:
        with (
            tc.tile_pool(name="kt_pool", bufs=2) as kt_pool,
            tc.tile_pool(name="va_pool", bufs=2) as va_pool,
            tc.tile_pool(name="qt_pool", bufs=2) as qt_pool,
            tc.tile_pool(name="e_pool", bufs=4) as e_pool,
            tc.tile_pool(name="ob_pool", bufs=8) as ob_pool,
            tc.tile_pool(name="rc_pool", bufs=8) as rc_pool,
            tc.tile_pool(name="s_pool", bufs=2, space="PSUM") as s_pool,
            tc.tile_pool(name="o_pool", bufs=1, space="PSUM") as o_pool,
        ):
            loaded = {}

            def load_unit(u):
                # qt first (every chunk needs it), then k/v interleaved in
                # eighths so chunk 0's compute starts after ~1/8 of the load
                qt = qt_pool.tile([128, QSPAN], BF16, name=f"qt_u{u}", tag="qt")
                nc.sync.dma_start(out=qt[:], in_=qt_ap[u])
                kt = kt_pool.tile([128, LK], BF16, name=f"kt_u{u}", tag="kt")
                va = va_pool.tile([128, NLC, VPAD], BF16,
                                  name=f"va_u{u}", tag="va")
                for eighth in range(8):
                    slk = bass.ts(eighth, LK // 8)
                    nc.sync.dma_start(out=kt[:, slk], in_=kt_ap[u][:, slk])
                    slv = bass.ts(eighth, NLC // 8)
                    nc.sync.dma_start(out=va[:, slv, :], in_=va_ap[u][:, slv, :])
                loaded[u] = (kt, va, qt)

            load_unit(0)
            for u in range(UNITS_PER_CORE):
                kt, va, qt = loaded.pop(u)

                # 3 PSUM banks hold 8 accumulators of [128 q, 129] (3+3+2)
                obank = [o_pool.tile([128, 512], F32, tag=f"ob{b}",
                                     name=f"obank{b}_u{u}")
                         for b in range(3)]

                # software-pipelined by one chunk: emit A(c) before B(c-1)
                # so PE's in-order queue runs A(c+1) while ACT(c) is busy
                # instead of stalling behind B(c)'s wait on ACT(c).
                etiles = {}
                for c in range(NLC + 1):
                    if c < NLC:
                        s = s_pool.tile([128, QSPAN], F32)
                        for half in range(2):
                            sl = bass.ts(half, 512)
                            nc.tensor.matmul(
                                s[:, sl], lhsT=kt[:, bass.ts(c, 128)],
                                rhs=qt[:, sl], start=True, stop=True)
                        e = e_pool.tile([128, QSPAN], BF16)
                        nc.scalar.activation(
                            e[:], s[:], mybir.ActivationFunctionType.Exp,
                            scale=SCALE)
                        etiles[c] = e
                    if c == 8 and u + 1 < UNITS_PER_CORE:
                        load_unit(u + 1)   # prefetch before output DMAs queue
                    if c == 0:
                        continue
                    e = etiles.pop(c - 1)
                    for j in range(NQB):
                        off = (j % 3) * 129
                        # start=True clears has_written for the WHOLE bank,
                        # so only the first region per bank may use it; the
                        # bank-wide clear leaves sibling regions' bits unset
                        # and their first start=False matmul overwrites.
                        nc.tensor.matmul(
                            obank[j // 3][:, off:off + 129],
                            lhsT=e[:, bass.ts(j, 128)],
                            rhs=va[:, c - 1, 0:129],
                            start=(c - 1 == 0 and j % 3 == 0),
                            stop=(c - 1 == NLC - 1))

                for j in range(NQB):
                    off = (j % 3) * 129
                    ob_ap = obank[j // 3]
                    recip = rc_pool.tile([128, 1], F32)
                    nc.vector.reciprocal(recip[:],
                                         ob_ap[:, off + 128:off + 129])
                    ot = ob_pool.tile([128, 128], F32)
                    nc.vector.tensor_scalar_mul(
                        ot[:], ob_ap[:, off:off + 128], recip[:])
                    nc.sync.dma_start(
                        out=out_ap[u][bass.ts(j, 128), :], in_=ot[:])

    nc.compile()
    return nc


def _get_program():
    global _CACHED
    if _CACHED is None:
        _CACHED = _build_program()
    return _CACHED


def _host_prep(q, k, v, frame_seqlen, current_block_start):
    fs = max(0, min(int(frame_seqlen), LK))
    bs = max(0, min(int(current_block_start), LK))
    w = np.ones(LK, np.float32)
    w[fs:bs] = math.exp(math.log(0.1))

    q = np.asarray(q, dtype=np.float32)
    k = np.asarray(k, dtype=np.float32)
    v = np.asarray(v, dtype=np.float32)

    qT = np.ascontiguousarray(q[0].transpose(1, 2, 0)).astype(NP_BF16)  # [H,128,LQ]
    kT = np.ascontiguousarray(k[0].transpose(1, 2, 0)).astype(NP_BF16)  # [H,128,LK]
    vA = np.zeros((H, LK, VPAD), np.float32)                            # [H,LK,256]
    vA[:, :, 0:D] = v[0].transpose(1, 0, 2) * w[None, :, None]
    vA[:, :, D] = w[None, :]
    vA = vA.astype(NP_BF16)

    in_maps = []
    for i in range(N_CORES):
        units = [3 * i + uu for uu in range(UNITS_PER_CORE)]
        heads = [g // 2 for g in units]
        qhs = [g % 2 for g in units]
        in_maps.append({
            "qt": np.ascontiguousarray(
                np.stack([qT[h, :, qh * QSPAN:(qh + 1) * QSPAN]
                          for h, qh in zip(heads, qhs)])),
            "kt": np.ascontiguousarray(np.stack([kT[h] for h in heads])),
            "va": np.ascontiguousarray(np.stack([vA[h] for h in heads])),
        })
    return in_maps


def _assemble(results):
    out = np.empty((B, LQ, H, D), np.float32)
    for i in range(N_CORES):
        o = results[i]["out"]  # [3, 1024, 128]
        for uu in range(UNITS_PER_CORE):
            g = 3 * i + uu
            h, qh = g // 2, g % 2
            out[0, qh * QSPAN:(qh + 1) * QSPAN, h, :] = o[uu]
    return out


def kernel(q, k, v, frame_seqlen, current_block_start):
    nc = _get_program()
    in_maps = _host_prep(q, k, v, frame_seqlen, current_block_start)
    res = run_bass_kernel_spmd(nc, in_maps, core_ids=list(range(N_CORES)))
    return _assemble(res.results)


# revision 18
# speedup vs baseline: 1.8687x; 1.8687x over previous
"""Trainium2 Bass kernel for CausalWanSelfAttention (KV-cache-bias attention).

Math: the reference's disjoint-segment attention + LSE merge is exactly
global softmax with a per-key bias b_l (log 0.1 on keys in
[frame_seqlen, current_block_start)).  exp needs no max-subtraction
(scores ~ N(0,1), max ~ 6), so out = (E @ V) / (1^T E) with
E = exp(scale*S + b_l) — the bias folds into the ACT exp as a
per-partition bias (partition = key index within the 128-chunk).

Sharding: 24 units = (head h in 0..11, q-half in {0,1}), 3 units per core.
Each unit: 1024 queries x 1 head x all 8192 keys, 64 key chunks of 128.

Device layout per unit (matmuls bf16, accumulate fp32 PSUM; all matmuls
stream 512 q-columns so PE runs long back-to-back bursts with only 3
ldweights per chunk — the v1 kernel's 8 stationary loads per chunk made
phase B ldweights-bound on HW):
  A:    S^T[l 128, q 1024] = kt-chunk^T @ qt          (1 ldw + 2 MM N=512)
  exp:  E = exp(S^T * scale + bias_l) bf16            (1 ACT instr)
  B:    O^T[d 128, q 1024] += v-chunk^T @ E           (1 ldw + 2 MM)
  norm: n[1, q 1024]      += ones^T @ E               (1 ldw + 2 MM)
Final divide by n and the [d,q]->[q,d] transpose happen host-side on the
fp32 partials (exact).
"""

import math
import sys

for _p in ("/opt/trn_rl_repo",):
    if _p not in sys.path:
        sys.path.insert(0, _p)

import numpy as np
import ml_dtypes

import concourse.bass as bass
import concourse.mybir as mybir
import concourse.tile as tile
from concourse import bacc
from concourse.bass_utils import run_bass_kernel_spmd

BF16 = mybir.dt.bfloat16
F32 = mybir.dt.float32
NP_BF16 = ml_dtypes.bfloat16

B, LQ, LK, H, D = 1, 2048, 8192, 12, 128
N_CORES = 8
UNITS_PER_CORE = 3          # 24 units = 12 heads x 2 q-halves
QSPAN = 1024                # queries per unit
NLC = LK // 128             # 64 key chunks of 128
SCALE = 1.0 / math.sqrt(D)

_CACHED = None


def _build_program():
    nc = bacc.Bacc("TRN2", target_bir_lowering=False, debug=False,
                   enable_asserts=False)

    qt_d = nc.dram_tensor("qt", [UNITS_PER_CORE, 128, QSPAN], BF16,
                          kind="ExternalInput")
    kt_d = nc.dram_tensor("kt", [UNITS_PER_CORE, 128, LK], BF16,
                          kind="ExternalInput")
    vl_d = nc.dram_tensor("vl", [UNITS_PER_CORE, LK, 128], BF16,
                          kind="ExternalInput")
    bias_d = nc.dram_tensor("bias", [128, NLC], F32, kind="ExternalInput")
    ot_d = nc.dram_tensor("ot", [UNITS_PER_CORE, 128, QSPAN], F32,
                          kind="ExternalOutput")
    nm_d = nc.dram_tensor("nm", [UNITS_PER_CORE, 1, QSPAN], F32,
                          kind="ExternalOutput")

    qt_ap = qt_d.ap()
    kt_ap = kt_d.ap()
    # [u, (c p), d] -> [u, p, c, d]: partition = key index within chunk
    vl_ap = vl_d.ap().rearrange("u (c p) d -> u p c d", p=128)
    bias_ap = bias_d.ap()
    ot_ap = ot_d.ap()
    nm_ap = nm_d.ap()

    with tile.TileContext(nc) as tc:
        with (
            tc.tile_pool(name="kt_pool", bufs=2) as kt_pool,
            tc.tile_pool(name="vl_pool", bufs=2) as vl_pool,
            tc.tile_pool(name="qt_pool", bufs=2) as qt_pool,
            tc.tile_pool(name="cn_pool", bufs=1) as cn_pool,
            tc.tile_pool(name="e_pool", bufs=4) as e_pool,
            tc.tile_pool(name="ob_pool", bufs=2) as ob_pool,
            tc.tile_pool(name="s_pool", bufs=2, space="PSUM") as s_pool,
            tc.tile_pool(name="o_pool", bufs=1, space="PSUM") as o_pool,
            tc.tile_pool(name="n_pool", bufs=1, space="PSUM") as n_pool,
        ):
            bias_t = cn_pool.tile([128, NLC], F32, name="bias_t")
            nc.sync.dma_start(out=bias_t[:], in_=bias_ap)
            ones_t = cn_pool.tile([128, 1], BF16, name="ones_t")
            nc.vector.memset(ones_t[:], 1.0)

            loaded = {}

            def load_unit(u):
                # qt first (every chunk needs it), then k/v interleaved in
                # eighths so chunk 0's compute starts after ~1/8 of the load
                qt = qt_pool.tile([128, QSPAN], BF16, name=f"qt_u{u}", tag="qt")
                nc.sync.dma_start(out=qt[:], in_=qt_ap[u])
                kt = kt_pool.tile([128, LK], BF16, name=f"kt_u{u}", tag="kt")
                vl = vl_pool.tile([128, NLC, 128], BF16,
                                  name=f"vl_u{u}", tag="vl")
                for eighth in range(8):
                    slk = bass.ts(eighth, LK // 8)
                    nc.sync.dma_start(out=kt[:, slk], in_=kt_ap[u][:, slk])
                    slv = bass.ts(eighth, NLC // 8)
                    nc.sync.dma_start(out=vl[:, slv, :], in_=vl_ap[u][:, slv, :])
                loaded[u] = (kt, vl, qt)

            load_unit(0)
            for u in range(UNITS_PER_CORE):
                kt, vl, qt = loaded.pop(u)

                ot = o_pool.tile([128, QSPAN], F32, name=f"ot_u{u}", tag="ot")
                nm = n_pool.tile([128, QSPAN], F32, name=f"nm_u{u}", tag="nm")

                # software-pipelined by one chunk: emit A(c) before B(c-1)
                # so PE's in-order queue runs A(c+1) while ACT(c) is busy
                # instead of stalling behind B(c)'s wait on ACT(c).
                etiles = {}
                for c in range(NLC + 1):
                    if c < NLC:
                        s = s_pool.tile([128, QSPAN], F32)
                        for half in range(2):
                            sl = bass.ts(half, 512)
                            nc.tensor.matmul(
                                s[:, sl], lhsT=kt[:, bass.ts(c, 128)],
                                rhs=qt[:, sl], start=True, stop=True)
                        e = e_pool.tile([128, QSPAN], BF16)
                        nc.scalar.activation(
                            e[:], s[:], mybir.ActivationFunctionType.Exp,
                            bias=bias_t[:, c:c + 1], scale=SCALE)
                        etiles[c] = e
                    if c == 8 and u + 1 < UNITS_PER_CORE:
                        load_unit(u + 1)   # prefetch before output DMAs queue
                    if c == 0:
                        continue
                    e = etiles.pop(c - 1)
                    for half in range(2):
                        sl = bass.ts(half, 512)
                        nc.tensor.matmul(
                            ot[:, sl], lhsT=vl[:, c - 1, :], rhs=e[:, sl],
                            start=(c - 1 == 0), stop=(c - 1 == NLC - 1))
                    for half in range(2):
                        sl = bass.ts(half, 512)
                        nc.tensor.matmul(
                            nm[0:1, sl], lhsT=ones_t[:], rhs=e[:, sl],
                            start=(c - 1 == 0), stop=(c - 1 == NLC - 1))

                ot_sb = ob_pool.tile([128, QSPAN], F32, name=f"otsb_u{u}",
                                     tag="otsb")
                nc.vector.tensor_scalar_add(ot_sb[:], ot[:], 0.0)
                nm_sb = ob_pool.tile([1, QSPAN], F32, name=f"nmsb_u{u}",
                                     tag="nmsb")
                nc.vector.tensor_scalar_add(nm_sb[:], nm[0:1, :], 0.0)
                nc.sync.dma_start(out=ot_ap[u], in_=ot_sb[:])
                nc.sync.dma_start(out=nm_ap[u], in_=nm_sb[:])

    nc.compile()
    return nc


def _get_program():
    global _CACHED
    if _CACHED is None:
        _CACHED = _build_program()
    return _CACHED


def _host_prep(q, k, v, frame_seqlen, current_block_start):
    fs = max(0, min(int(frame_seqlen), LK))
    bs = max(0, min(int(current_block_start), LK))
    logw = np.zeros(LK, np.float32)
    logw[fs:bs] = math.log(0.1)
    bias = np.ascontiguousarray(logw.reshape(NLC, 128).T)  # [128, NLC]

    q = np.asarray(q, dtype=np.float32)
    k = np.asarray(k, dtype=np.float32)
    v = np.asarray(v, dtype=np.float32)

    qT = np.ascontiguousarray(q[0].transpose(1, 2, 0)).astype(NP_BF16)  # [H,128,LQ]
    kT = np.ascontiguousarray(k[0].transpose(1, 2, 0)).astype(NP_BF16)  # [H,128,LK]
    vL = np.ascontiguousarray(v[0].transpose(1, 0, 2)).astype(NP_BF16)  # [H,LK,128]

    in_maps = []
    for i in range(N_CORES):
        units = [3 * i + uu for uu in range(UNITS_PER_CORE)]
        heads = [g // 2 for g in units]
        qhs = [g % 2 for g in units]
        in_maps.append({
            "qt": np.ascontiguousarray(
                np.stack([qT[h, :, qh * QSPAN:(qh + 1) * QSPAN]
                          for h, qh in zip(heads, qhs)])),
            "kt": np.ascontiguousarray(np.stack([kT[h] for h in heads])),
            "vl": np.ascontiguousarray(np.stack([vL[h] for h in heads])),
            "bias": bias,
        })
    return in_maps


def _assemble(results):
    out = np.empty((B, LQ, H, D), np.float32)
    for i in range(N_CORES):
        ot = results[i]["ot"]   # [3, 128, 1024] unnormalized O^T
        nm = results[i]["nm"][:, 0]   # [3, 1024]
        for uu in range(UNITS_PER_CORE):
            g = 3 * i + uu
            h, qh = g // 2, g % 2
            out[0, qh * QSPAN:(qh + 1) * QSPAN, h, :] = (
                ot[uu] / nm[uu][None, :]).T
    return out


def kernel(q, k, v, frame_seqlen, current_block_start):
    nc = _get_program()
    in_maps = _host_prep(q, k, v, frame_seqlen, current_block_start)
    res = run_bass_kernel_spmd(nc, in_maps, core_ids=list(range(N_CORES)))
    return _assemble(res.results)
